# revision 5
# baseline (speedup 1.0000x reference)
"""Distributed Trainium2 kernel for the focus-present sparse attention module.

Semantics (B=2, N=2048, DIM=256, H=4, DH=32):
    qkv = x @ W_qkv ; q,k,v split into H heads of DH
    sim = q@k^T * DH^-0.5 + pos_bias ; batches with focus_present_mask=True
    attend only to self (softmax over a single unmasked logit == identity),
    so their output is exactly v @ W_out. Unmasked batches do full softmax
    attention with the additive [H,N,N] pos_bias.

Strategy: inspect the mask on host and dispatch to a graph compiled for
that mask pattern (cached, one compile per pattern per process). Work is
sharded by query rows: core i owns rows [i*256, (i+1)*256) of every batch,
so the output shards are disjoint and no collective is needed, and each
element of pos_bias is read exactly once across the chip (the memory
roofline for this problem).

Per batch on each core:
  masked:   at^T = Wv^T @ x_chunk^T            (identity attention)
  unmasked: q^T/k^T/v from x^T (transposed layout keeps every matmul's
            contraction dim on partitions with no on-device transposes),
            sim^T = k^T(tile)·q, + pos_bias^T tile (streamed), exp on
            ScalarE, attn·v via PE with an appended ones-column in v that
            yields the softmax normalizer for free, then one reciprocal +
            broadcast-multiply.
  epilogue: out_rows = (at^T)^T @ W_out, DMA to the core's output slice.

Host-side numpy only slices/transposes inputs (no FLOPs on the answer).
"""

import numpy as np

import concourse.bacc as bacc
import concourse.mybir as mybir
import concourse.tile as tile
from concourse.bass_utils import run_bass_kernel_spmd

B, N, DIM, H, DH = 2, 2048, 256, 4, 32
NCORES = 8
RPC = N // NCORES  # 256 query rows per core per batch
NKT = N // 128  # 16 key tiles
HD = H * DH  # 128
VW = DH + 1  # per-head v block width incl. ones column
SIMW = H * RPC  # 1024: sim tile free width, (head, q) packed

f32 = mybir.dt.float32
bf16 = mybir.dt.bfloat16

_graph_cache: dict = {}


def _build(mask):
    unmasked = [b for b in range(B) if not mask[b]]
    n_u = len(unmasked)

    nc = bacc.Bacc(None, target_bir_lowering=False)

    xq_p = nc.declare_dram_parameter("xq", [DIM, B * RPC], f32, isOutput=False)
    wvall_p = nc.declare_dram_parameter("wvall", [DIM, HD], f32, isOutput=False)
    wout_p = nc.declare_dram_parameter("wout", [HD, DIM], f32, isOutput=False)
    out_p = nc.declare_dram_parameter("out", [B * RPC, DIM], f32, isOutput=True)
    if n_u:
        xtu_p = nc.declare_dram_parameter("xtu", [DIM, n_u * N], f32, isOutput=False)
        wqall_p = nc.declare_dram_parameter("wqall", [DIM, HD], f32, isOutput=False)
        wkall_p = nc.declare_dram_parameter("wkall", [DIM, HD], f32, isOutput=False)
        post_p = nc.declare_dram_parameter("post", [N, SIMW], f32, isOutput=False)

    with tile.TileContext(nc) as tc:
        with (
            tc.tile_pool(name="w", bufs=1) as wpool,
            tc.tile_pool(name="io", bufs=3) as iopool,
            tc.tile_pool(name="big", bufs=1) as bigpool,
            tc.tile_pool(name="pos", bufs=3) as pospool,
            tc.tile_pool(name="mid", bufs=3) as midpool,
            tc.tile_pool(name="ps", bufs=2, space="PSUM") as pspool,
            tc.tile_pool(name="psav", bufs=1, space="PSUM") as avpool,
        ):
            def load_halves(param, cols, tagbase):
                halves = []
                for kk in range(2):
                    t = wpool.tile([128, cols], f32, tag=f"{tagbase}{kk}")
                    nc.sync.dma_start(t[:], param[kk * 128 : (kk + 1) * 128, :])
                    halves.append(t)
                return halves

            wout_sb = wpool.tile([HD, DIM], f32, tag="wout")
            nc.sync.dma_start(wout_sb[:], wout_p[:])
            wvall_sb = load_halves(wvall_p, HD, "wvall")
            xq_sb = load_halves(xq_p, B * RPC, "xq")

            if n_u:
                wqall_sb = load_halves(wqall_p, HD, "wqall")
                wkall_sb = load_halves(wkall_p, HD, "wkall")
                ones_sb = wpool.tile([1, DH], f32, tag="ones")
                nc.vector.memset(ones_sb[:], 1.0)

                # per unmasked batch: x^T resident, q^T/k^T/v projections
                xus, qts, kts, vs, avs = [], [], [], [], []
                for j in range(n_u):
                    xu = []
                    for kk in range(2):
                        t = bigpool.tile([128, N], f32, tag=f"xu{j}{kk}")
                        nc.sync.dma_start(
                            t[:],
                            xtu_p[kk * 128 : (kk + 1) * 128, j * N : (j + 1) * N],
                        )
                        xu.append(t)
                    xus.append(xu)

                    b = unmasked[j]
                    qt_ps = pspool.tile([HD, RPC], f32, tag="ps_big")
                    for kk in range(2):
                        nc.tensor.matmul(
                            qt_ps[:],
                            wqall_sb[kk][:],
                            xq_sb[kk][:, b * RPC : (b + 1) * RPC],
                            start=(kk == 0),
                            stop=(kk == 1),
                        )
                    qt_sb = bigpool.tile([HD, RPC], bf16, tag=f"qt{j}")
                    nc.vector.tensor_copy(qt_sb[:], qt_ps[:])
                    qts.append(qt_sb)

                    kt_sb = bigpool.tile([HD, N], bf16, tag=f"kt{j}")
                    for w in range(N // 512):
                        kt_ps = pspool.tile([HD, 512], f32, tag="ps_big")
                        for kk in range(2):
                            nc.tensor.matmul(
                                kt_ps[:],
                                wkall_sb[kk][:],
                                xu[kk][:, w * 512 : (w + 1) * 512],
                                start=(kk == 0),
                                stop=(kk == 1),
                            )
                        nc.vector.tensor_copy(
                            kt_sb[:, w * 512 : (w + 1) * 512], kt_ps[:]
                        )
                    kts.append(kt_sb)

                    # v natural [row, ch], head-major blocks of (DH | ones)
                    v_sb = bigpool.tile([128, NKT * H * VW], bf16, tag=f"v{j}")
                    nc.vector.memset(v_sb[:], 1.0)
                    for t in range(NKT):
                        v_ps = pspool.tile([128, HD], f32, tag="ps_big")
                        for kk in range(2):
                            nc.tensor.matmul(
                                v_ps[:],
                                xu[kk][:, t * 128 : (t + 1) * 128],
                                wvall_sb[kk][:],
                                start=(kk == 0),
                                stop=(kk == 1),
                            )
                        dst = (
                            v_sb[:, t * H * VW : (t + 1) * H * VW]
                            .rearrange("p (h w) -> p h w", h=H)[:, :, 0:DH]
                        )
                        nc.vector.tensor_copy(dst, v_ps[:])
                    vs.append(v_sb)
                    avs.append(avpool.tile([VW, SIMW], f32, tag=f"av{j}"))

                # main loop: stream pos_bias^T once, use for every batch
                for t in range(NKT):
                    post_sb = pospool.tile([128, SIMW], f32, tag="post")
                    nc.sync.dma_start(post_sb[:], post_p[t * 128 : (t + 1) * 128, :])
                    for j in range(n_u):
                        sim_ps = pspool.tile([128, SIMW], f32, tag="ps_big")
                        for h in range(H):
                            nc.tensor.matmul(
                                sim_ps[:, h * RPC : (h + 1) * RPC],
                                kts[j][h * DH : (h + 1) * DH, t * 128 : (t + 1) * 128],
                                qts[j][h * DH : (h + 1) * DH, :],
                                start=True,
                                stop=True,
                            )
                        simb_sb = midpool.tile([128, SIMW], f32, tag="simb")
                        nc.vector.tensor_add(simb_sb[:], sim_ps[:], post_sb[:])
                        exp_sb = midpool.tile([128, SIMW], bf16, tag="exp")
                        nc.scalar.activation(
                            exp_sb[:], simb_sb[:], mybir.ActivationFunctionType.Exp
                        )
                        for h in range(H):
                            nc.tensor.matmul(
                                avs[j][:, h * RPC : (h + 1) * RPC],
                                vs[j][
                                    :,
                                    t * H * VW + h * VW : t * H * VW + (h + 1) * VW,
                                ],
                                exp_sb[:, h * RPC : (h + 1) * RPC],
                                start=(t == 0),
                                stop=(t == NKT - 1),
                            )

            # epilogue per batch: at^T [HD, RPC] -> out rows
            for b in range(B):
                at_sb = iopool.tile([HD, RPC], f32, tag="at")
                if mask[b]:
                    vt_ps = pspool.tile([HD, RPC], f32, tag="ps_big")
                    for kk in range(2):
                        nc.tensor.matmul(
                            vt_ps[:],
                            wvall_sb[kk][:],
                            xq_sb[kk][:, b * RPC : (b + 1) * RPC],
                            start=(kk == 0),
                            stop=(kk == 1),
                        )
                    nc.vector.tensor_copy(at_sb[:], vt_ps[:])
                else:
                    j = unmasked.index(b)
                    recip_sb = midpool.tile([1, SIMW], f32, tag="recip")
                    nc.vector.reciprocal(recip_sb[:], avs[j][DH : DH + 1, :])
                    bc_ps = pspool.tile([DH, SIMW], f32, tag="ps_big")
                    for w in range(SIMW // 512):
                        nc.tensor.matmul(
                            bc_ps[:, w * 512 : (w + 1) * 512],
                            ones_sb[:],
                            recip_sb[:, w * 512 : (w + 1) * 512],
                            start=True,
                            stop=True,
                        )
                    bc_sb = midpool.tile([DH, SIMW], f32, tag="bc")
                    nc.vector.tensor_copy(bc_sb[:], bc_ps[:])
                    for h in range(H):
                        nc.vector.tensor_mul(
                            at_sb[h * DH : (h + 1) * DH, :],
                            avs[j][0:DH, h * RPC : (h + 1) * RPC],
                            bc_sb[:, h * RPC : (h + 1) * RPC],
                        )
                for half in range(RPC // 128):
                    o_ps = pspool.tile([128, DIM], f32, tag="ps_big")
                    nc.tensor.matmul(
                        o_ps[:],
                        at_sb[:, half * 128 : (half + 1) * 128],
                        wout_sb[:],
                        start=True,
                        stop=True,
                    )
                    o_sb = iopool.tile([128, DIM], f32, tag="om")
                    nc.vector.tensor_copy(o_sb[:], o_ps[:])
                    nc.sync.dma_start(
                        out_p[b * RPC + half * 128 : b * RPC + (half + 1) * 128, :],
                        o_sb[:],
                    )

    nc.compile()
    return nc


def _prepare_in_maps(mask, x, pos_bias, W_qkv, W_out):
    unmasked = [b for b in range(B) if not mask[b]]
    scale = np.float32(DH**-0.5)

    xT = [np.ascontiguousarray(x[b].T) for b in range(B)]  # [DIM, N]
    wout = np.ascontiguousarray(W_out)
    wvall = np.ascontiguousarray(W_qkv[:, 2 * HD :])
    if unmasked:
        wqall = np.ascontiguousarray(W_qkv[:, 0:HD] * scale)
        wkall = np.ascontiguousarray(W_qkv[:, HD : 2 * HD])
        xtu = np.concatenate([xT[b] for b in unmasked], axis=1)
        # post_full[k, h, q] = pos_bias[h, q, k]
        post_full = np.ascontiguousarray(pos_bias.transpose(2, 0, 1))

    in_maps = []
    for core in range(NCORES):
        m = {
            "wout": wout,
            "wvall": wvall,
            "xq": np.ascontiguousarray(
                np.concatenate(
                    [xT[b][:, core * RPC : (core + 1) * RPC] for b in range(B)], axis=1
                )
            ),
        }
        if unmasked:
            m["xtu"] = xtu
            m["wqall"] = wqall
            m["wkall"] = wkall
            m["post"] = np.ascontiguousarray(
                post_full[:, :, core * RPC : (core + 1) * RPC].reshape(N, SIMW)
            )
        in_maps.append(m)
    return in_maps


def kernel(x, pos_bias, focus_present_mask, W_qkv, W_out):
    x = np.asarray(x, dtype=np.float32)
    pos_bias = np.asarray(pos_bias, dtype=np.float32)
    focus_present_mask = np.asarray(focus_present_mask).astype(bool)
    W_qkv = np.asarray(W_qkv, dtype=np.float32)
    W_out = np.asarray(W_out, dtype=np.float32)

    mask = tuple(bool(v) for v in focus_present_mask)
    if mask not in _graph_cache:
        _graph_cache[mask] = _build(mask)
    nc = _graph_cache[mask]

    in_maps = _prepare_in_maps(mask, x, pos_bias, W_qkv, W_out)
    res = run_bass_kernel_spmd(nc, in_maps, core_ids=list(range(NCORES)))
    global _last_exec_ns
    _last_exec_ns = res.exec_time_ns

    out = np.empty((B, N, DIM), dtype=np.float32)
    for core in range(NCORES):
        blk = res.results[core]["out"]
        for b in range(B):
            out[b, core * RPC : (core + 1) * RPC] = blk[b * RPC : (b + 1) * RPC]
    return out


# revision 9
# speedup vs baseline: 1.0173x; 1.0173x over previous
"""Distributed Trainium2 kernel for the focus-present sparse attention module.

Semantics (B=2, N=2048, DIM=256, H=4, DH=32):
    qkv = x @ W_qkv ; q,k,v split into H heads of DH
    sim = q@k^T * DH^-0.5 + pos_bias ; batches with focus_present_mask=True
    attend only to self (softmax over a single unmasked logit == identity),
    so their output is exactly v @ W_out. Unmasked batches do full softmax
    attention with the additive [H,N,N] pos_bias.

Strategy: inspect the mask on host and dispatch to a graph compiled for
that mask pattern (cached, one compile per pattern per process). Work is
sharded by query rows: core i owns rows [i*256, (i+1)*256) of every batch,
so the output shards are disjoint and no collective is needed, and each
element of pos_bias is read exactly once across the chip (the memory
roofline for this problem).

Per batch on each core:
  masked:   at^T = Wv^T @ x_chunk^T            (identity attention)
  unmasked: q^T/k^T/v from x^T (transposed layout keeps every matmul's
            contraction dim on partitions with no on-device transposes),
            sim^T = k^T(tile)·q, + pos_bias^T tile (streamed), exp on
            ScalarE, attn·v via PE with an appended ones-column in v that
            yields the softmax normalizer for free, then one reciprocal +
            broadcast-multiply.
  epilogue: out_rows = (at^T)^T @ W_out, DMA to the core's output slice.

Host-side numpy only slices/transposes inputs (no FLOPs on the answer).
"""

import numpy as np

import concourse.bacc as bacc
import concourse.mybir as mybir
import concourse.tile as tile
from concourse.bass_utils import run_bass_kernel_spmd

B, N, DIM, H, DH = 2, 2048, 256, 4, 32
NCORES = 8
RPC = N // NCORES  # 256 query rows per core per batch
NKT = N // 128  # 16 key tiles
HD = H * DH  # 128
VW = DH + 1  # per-head v block width incl. ones column
SIMW = H * RPC  # 1024: sim tile free width, (head, q) packed

f32 = mybir.dt.float32
bf16 = mybir.dt.bfloat16

_graph_cache: dict = {}


def _build(mask):
    unmasked = [b for b in range(B) if not mask[b]]
    n_u = len(unmasked)

    nc = bacc.Bacc(None, target_bir_lowering=False)

    xq_p = nc.declare_dram_parameter("xq", [DIM, B * RPC], f32, isOutput=False)
    wvall_p = nc.declare_dram_parameter("wvall", [DIM, HD], f32, isOutput=False)
    wout_p = nc.declare_dram_parameter("wout", [HD, DIM], f32, isOutput=False)
    out_p = nc.declare_dram_parameter("out", [B * RPC, DIM], f32, isOutput=True)
    if n_u:
        xtu_p = nc.declare_dram_parameter("xtu", [DIM, n_u * N], f32, isOutput=False)
        wqall_p = nc.declare_dram_parameter("wqall", [DIM, HD], f32, isOutput=False)
        wkall_p = nc.declare_dram_parameter("wkall", [DIM, HD], f32, isOutput=False)
        post_p = nc.declare_dram_parameter("post", [N, SIMW], f32, isOutput=False)

    with tile.TileContext(nc) as tc:
        with (
            tc.tile_pool(name="w", bufs=1) as wpool,
            tc.tile_pool(name="io", bufs=3) as iopool,
            tc.tile_pool(name="big", bufs=1) as bigpool,
            tc.tile_pool(name="pos", bufs=3) as pospool,
            tc.tile_pool(name="mid", bufs=3) as midpool,
            tc.tile_pool(name="ps", bufs=2, space="PSUM") as pspool,
            tc.tile_pool(name="psav", bufs=1, space="PSUM") as avpool,
        ):
            def load_halves(param, cols, tagbase):
                halves = []
                for kk in range(2):
                    t = wpool.tile([128, cols], f32, tag=f"{tagbase}{kk}")
                    nc.sync.dma_start(t[:], param[kk * 128 : (kk + 1) * 128, :])
                    halves.append(t)
                return halves

            wout_sb = wpool.tile([HD, DIM], f32, tag="wout")
            nc.sync.dma_start(wout_sb[:], wout_p[:])
            wvall_sb = load_halves(wvall_p, HD, "wvall")
            xq_sb = load_halves(xq_p, B * RPC, "xq")

            if n_u:
                wqall_sb = load_halves(wqall_p, HD, "wqall")
                wkall_sb = load_halves(wkall_p, HD, "wkall")
                ones_sb = wpool.tile([128, 1], bf16, tag="ones")
                nc.vector.memset(ones_sb[:], 1.0)
                one1_sb = wpool.tile([1, DH], f32, tag="one1")
                nc.vector.memset(one1_sb[:], 1.0)

                # per unmasked batch: x^T resident, q^T/k^T/v projections
                xus, qts, kts, vs, avs, exps = [], [], [], [], [], []
                for j in range(n_u):
                    xu = []
                    for kk in range(2):
                        t = bigpool.tile([128, N], f32, tag=f"xu{j}{kk}")
                        nc.sync.dma_start(
                            t[:],
                            xtu_p[kk * 128 : (kk + 1) * 128, j * N : (j + 1) * N],
                        )
                        xu.append(t)
                    xus.append(xu)

                    b = unmasked[j]
                    qt_ps = pspool.tile([HD, RPC], f32, tag="ps_big")
                    for kk in range(2):
                        nc.tensor.matmul(
                            qt_ps[:],
                            wqall_sb[kk][:],
                            xq_sb[kk][:, b * RPC : (b + 1) * RPC],
                            start=(kk == 0),
                            stop=(kk == 1),
                        )
                    qt_sb = [
                        bigpool.tile([DH, RPC], bf16, tag=f"qt{j}h{h}", name=f"qt{j}h{h}")
                        for h in range(H)
                    ]
                    for h in range(H):
                        nc.vector.tensor_copy(
                            qt_sb[h][:], qt_ps[h * DH : (h + 1) * DH, :]
                        )
                    qts.append(qt_sb)

                    kt_sb = [
                        bigpool.tile([DH, N], bf16, tag=f"kt{j}h{h}", name=f"kt{j}h{h}")
                        for h in range(H)
                    ]
                    for w in range(N // 512):
                        kt_ps = pspool.tile([HD, 512], f32, tag="ps_big")
                        for kk in range(2):
                            nc.tensor.matmul(
                                kt_ps[:],
                                wkall_sb[kk][:],
                                xu[kk][:, w * 512 : (w + 1) * 512],
                                start=(kk == 0),
                                stop=(kk == 1),
                            )
                        for h in range(H):
                            nc.vector.tensor_copy(
                                kt_sb[h][:, w * 512 : (w + 1) * 512],
                                kt_ps[h * DH : (h + 1) * DH, :],
                            )
                    kts.append(kt_sb)

                    # v natural [row, ch]
                    v_sb = bigpool.tile([128, NKT * HD], bf16, tag=f"v{j}")
                    for t in range(NKT):
                        v_ps = pspool.tile([128, HD], f32, tag="ps_big")
                        for kk in range(2):
                            nc.tensor.matmul(
                                v_ps[:],
                                xu[kk][:, t * 128 : (t + 1) * 128],
                                wvall_sb[kk][:],
                                start=(kk == 0),
                                stop=(kk == 1),
                            )
                        nc.vector.tensor_copy(
                            v_sb[:, t * HD : (t + 1) * HD], v_ps[:]
                        )
                    vs.append(v_sb)
                    # full [v-channel x (head,q)] accumulator; only the
                    # diagonal head blocks are used, but a single matmul per
                    # tile costs the same and keeps one clean PSUM group
                    avs.append(
                        avpool.tile([HD, SIMW], f32, tag=f"av{j}", name=f"av{j}")
                    )
                    exps.append(
                        [
                            bigpool.tile(
                                [128, SIMW], bf16, tag=f"exp{j}t{t}", name=f"exp{j}t{t}"
                            )
                            for t in range(NKT)
                        ]
                    )

                # main loop: stream pos_bias^T once, use for every batch
                for t in range(NKT):
                    post_sb = pospool.tile([128, SIMW], f32, tag="post")
                    nc.sync.dma_start(post_sb[:], post_p[t * 128 : (t + 1) * 128, :])
                    for j in range(n_u):
                        sim_ps = pspool.tile([128, SIMW], f32, tag="ps_big")
                        for h in range(H):
                            nc.tensor.matmul(
                                sim_ps[:, h * RPC : (h + 1) * RPC],
                                kts[j][h][:, t * 128 : (t + 1) * 128],
                                qts[j][h][:],
                                start=True,
                                stop=True,
                            )
                        simb_sb = midpool.tile([128, SIMW], f32, tag="simb")
                        nc.vector.tensor_add(simb_sb[:], sim_ps[:], post_sb[:])
                        exp_sb = exps[j][t]
                        nc.scalar.activation(
                            exp_sb[:], simb_sb[:], mybir.ActivationFunctionType.Exp
                        )
                        for w in range(SIMW // 512):
                            nc.tensor.matmul(
                                avs[j][:, w * 512 : (w + 1) * 512],
                                vs[j][:, t * HD : (t + 1) * HD],
                                exp_sb[:, w * 512 : (w + 1) * 512],
                                start=(t == 0),
                                stop=(t == NKT - 1),
                            )

            # epilogue per batch: at^T [HD, RPC] -> out rows
            for b in range(B):
                at_sb = iopool.tile([HD, RPC], f32, tag="at")
                if mask[b]:
                    vt_ps = pspool.tile([HD, RPC], f32, tag="ps_big")
                    for kk in range(2):
                        nc.tensor.matmul(
                            vt_ps[:],
                            wvall_sb[kk][:],
                            xq_sb[kk][:, b * RPC : (b + 1) * RPC],
                            start=(kk == 0),
                            stop=(kk == 1),
                        )
                    nc.vector.tensor_copy(at_sb[:], vt_ps[:])
                else:
                    j = unmasked.index(b)
                    cs_ps = pspool.tile([1, SIMW], f32, tag="ps_big")
                    for t in range(NKT):
                        for w in range(SIMW // 512):
                            nc.tensor.matmul(
                                cs_ps[:, w * 512 : (w + 1) * 512],
                                ones_sb[:],
                                exps[j][t][:, w * 512 : (w + 1) * 512],
                                start=(t == 0),
                                stop=(t == NKT - 1),
                            )
                    recip_sb = midpool.tile([1, SIMW], f32, tag="recip")
                    nc.vector.reciprocal(recip_sb[:], cs_ps[:])
                    bc_ps = pspool.tile([DH, SIMW], f32, tag="ps_big")
                    for w in range(SIMW // 512):
                        nc.tensor.matmul(
                            bc_ps[:, w * 512 : (w + 1) * 512],
                            one1_sb[:],
                            recip_sb[:, w * 512 : (w + 1) * 512],
                            start=True,
                            stop=True,
                        )
                    bc_sb = midpool.tile([DH, SIMW], f32, tag="bc")
                    nc.vector.tensor_copy(bc_sb[:], bc_ps[:])
                    for h in range(H):
                        nc.vector.tensor_mul(
                            at_sb[h * DH : (h + 1) * DH, :],
                            avs[j][h * DH : (h + 1) * DH, h * RPC : (h + 1) * RPC],
                            bc_sb[:, h * RPC : (h + 1) * RPC],
                        )
                for half in range(RPC // 128):
                    o_ps = pspool.tile([128, DIM], f32, tag="ps_big")
                    nc.tensor.matmul(
                        o_ps[:],
                        at_sb[:, half * 128 : (half + 1) * 128],
                        wout_sb[:],
                        start=True,
                        stop=True,
                    )
                    o_sb = iopool.tile([128, DIM], f32, tag="om")
                    nc.vector.tensor_copy(o_sb[:], o_ps[:])
                    nc.sync.dma_start(
                        out_p[b * RPC + half * 128 : b * RPC + (half + 1) * 128, :],
                        o_sb[:],
                    )

    nc.compile()
    return nc


def _prepare_in_maps(mask, x, pos_bias, W_qkv, W_out):
    unmasked = [b for b in range(B) if not mask[b]]
    scale = np.float32(DH**-0.5)

    xT = [np.ascontiguousarray(x[b].T) for b in range(B)]  # [DIM, N]
    wout = np.ascontiguousarray(W_out)
    wvall = np.ascontiguousarray(W_qkv[:, 2 * HD :])
    if unmasked:
        wqall = np.ascontiguousarray(W_qkv[:, 0:HD] * scale)
        wkall = np.ascontiguousarray(W_qkv[:, HD : 2 * HD])
        xtu = np.concatenate([xT[b] for b in unmasked], axis=1)
        # post_full[k, h, q] = pos_bias[h, q, k]
        post_full = np.ascontiguousarray(pos_bias.transpose(2, 0, 1))

    in_maps = []
    for core in range(NCORES):
        m = {
            "wout": wout,
            "wvall": wvall,
            "xq": np.ascontiguousarray(
                np.concatenate(
                    [xT[b][:, core * RPC : (core + 1) * RPC] for b in range(B)], axis=1
                )
            ),
        }
        if unmasked:
            m["xtu"] = xtu
            m["wqall"] = wqall
            m["wkall"] = wkall
            m["post"] = np.ascontiguousarray(
                post_full[:, :, core * RPC : (core + 1) * RPC].reshape(N, SIMW)
            )
        in_maps.append(m)
    return in_maps


def kernel(x, pos_bias, focus_present_mask, W_qkv, W_out):
    x = np.asarray(x, dtype=np.float32)
    pos_bias = np.asarray(pos_bias, dtype=np.float32)
    focus_present_mask = np.asarray(focus_present_mask).astype(bool)
    W_qkv = np.asarray(W_qkv, dtype=np.float32)
    W_out = np.asarray(W_out, dtype=np.float32)

    mask = tuple(bool(v) for v in focus_present_mask)
    if mask not in _graph_cache:
        _graph_cache[mask] = _build(mask)
    nc = _graph_cache[mask]

    in_maps = _prepare_in_maps(mask, x, pos_bias, W_qkv, W_out)
    res = run_bass_kernel_spmd(nc, in_maps, core_ids=list(range(NCORES)))
    global _last_exec_ns
    _last_exec_ns = res.exec_time_ns

    out = np.empty((B, N, DIM), dtype=np.float32)
    for core in range(NCORES):
        blk = res.results[core]["out"]
        for b in range(B):
            out[b, core * RPC : (core + 1) * RPC] = blk[b * RPC : (b + 1) * RPC]
    return out
